# revision 7
# baseline (speedup 1.0000x reference)
"""Batched rule-expert FFN (MoE routing) on 8 Trainium2 NeuronCores.

Strategy (expert/slot parallel with host-side dispatch):
  - Sort tokens by rule id on the host; each rule's tokens form one "slot"
    (rules with more than `Cr` tokens get several slots, zero-hit rules get
    an empty slot so the device schedule stays fully static).
  - Slots are dealt contiguously to the 8 cores (128 slots/core for 1024
    rules).  For each core the host gathers that core's slot weights
    w1[rule], w2[rule], biases, plus an x^T buffer [128, K_c*Cr] whose
    column block k*Cr:(k+1)*Cr holds the (transposed, zero-padded) tokens
    of slot k.
  - The device kernel is a static loop over slot-blocks: load w1/w2/bias
    blocks, per slot run  H^T = gelu(W1^T X^T + b1),  Out^T = W2^T H^T + b2
    with tokens in the free (moving) dimension, biases folded into the
    PSUM accumulation as K=1 matmuls against a ones row.
  - Host scatters Out^T columns back to token order.

Traffic per core ~= 32 MiB of expert tables (+ ~4 MiB padded x/out), which
is the memory roofline for this problem: every rule is hit with very high
probability, so the whole [R,D,E]+[R,E,D] table must be read exactly once.
"""

import numpy as np

import concourse.bass as bass
import concourse.bacc as bacc
import concourse.mybir as mybir
from concourse.tile import TileContext
from concourse.bass_utils import run_bass_kernel_spmd

N_CORES = 8
D = 128   # d_model
E = 256   # expert dim
EC = E // 128  # e-chunks of 128 partitions


def _build_nc(K_c: int, Cr: int, B: int):
    """Bass program for one core: K_c slots of capacity Cr, B slots/block."""
    f32 = mybir.dt.float32
    nc = bacc.Bacc("TRN2", target_bir_lowering=False)

    xT = nc.declare_dram_parameter("xT", [D, K_c * Cr], f32, isOutput=False)
    w1 = nc.declare_dram_parameter("w1", [K_c, D, E], f32, isOutput=False)
    w2 = nc.declare_dram_parameter("w2", [K_c, E, D], f32, isOutput=False)
    bb = nc.declare_dram_parameter("bias", [K_c, E + D], f32, isOutput=False)
    outT = nc.declare_dram_parameter("outT", [D, K_c * Cr], f32, isOutput=True)

    bbv = bb.rearrange("r f -> (r f)")                      # [K_c*384]

    nblk = K_c // B
    gelu = mybir.ActivationFunctionType.Gelu

    with TileContext(nc) as tc:
        with (
            tc.tile_pool(name="wpool", bufs=2) as wpool,
            tc.tile_pool(name="xpool", bufs=3) as xpool,
            tc.tile_pool(name="bpool", bufs=3) as bpool,
            tc.tile_pool(name="hpool", bufs=2) as hpool,
            tc.tile_pool(name="opool", bufs=3) as opool,
            tc.tile_pool(name="ones_pool", bufs=1) as ones_pool,
            tc.tile_pool(name="ppool", bufs=2, space="PSUM") as ppool,
        ):
            ones = ones_pool.tile([1, Cr], f32)
            nc.vector.memset(ones, 1.0)

            for j in range(nblk):
                w1t = wpool.tile([128, B * E], f32, tag="w1t")
                nc.sync.dma_start(
                    out=w1t.rearrange("p (r e) -> p r e", e=E),
                    in_=w1[j * B:(j + 1) * B].rearrange("r d e -> d r e"))
                w2t = wpool.tile([128, B * E], f32, tag="w2t")
                nc.sync.dma_start(
                    out=w2t.rearrange("p (r c d) -> p r c d", c=EC, d=128),
                    in_=w2[j * B:(j + 1) * B].rearrange(
                        "r (c p) d -> p r c d", p=128))
                bt = bpool.tile([1, B * (E + D)], f32, tag="bt")
                nc.sync.dma_start(
                    out=bt,
                    in_=bbv[None, j * B * (E + D):(j + 1) * B * (E + D)])
                xt = xpool.tile([128, B * Cr], f32, tag="xt")
                nc.sync.dma_start(
                    out=xt, in_=xT[:, j * B * Cr:(j + 1) * B * Cr])

                ph0 = ppool.tile([128, B * Cr], f32, tag="ph0")
                ph1 = ppool.tile([128, B * Cr], f32, tag="ph1")
                po = ppool.tile([128, B * Cr], f32, tag="po")

                # ---- layer 1: H^T[e, tok] = W1^T X^T + b1 ---------------
                for b in range(B):
                    cs = slice(b * Cr, (b + 1) * Cr)
                    nc.tensor.matmul(
                        ph0[:, cs], lhsT=w1t[:, b * E:b * E + 128],
                        rhs=xt[:, cs], start=True, stop=False)
                    nc.tensor.matmul(
                        ph0[:, cs], lhsT=bt[0:1, b * (E + D):b * (E + D) + 128],
                        rhs=ones[0:1, :], start=False, stop=True)
                    nc.tensor.matmul(
                        ph1[:, cs], lhsT=w1t[:, b * E + 128:b * E + 256],
                        rhs=xt[:, cs], start=True, stop=False)
                    nc.tensor.matmul(
                        ph1[:, cs], lhsT=bt[0:1, b * (E + D) + 128:b * (E + D) + 256],
                        rhs=ones[0:1, :], start=False, stop=True)

                # ---- gelu (exact erf flavor) ----------------------------
                h0 = hpool.tile([128, B * Cr], f32, tag="h0")
                nc.scalar.activation(h0, ph0, gelu)
                h1 = hpool.tile([128, B * Cr], f32, tag="h1")
                nc.scalar.activation(h1, ph1, gelu)

                # ---- layer 2: Out^T[d, tok] = W2^T H^T + b2 -------------
                for b in range(B):
                    cs = slice(b * Cr, (b + 1) * Cr)
                    nc.tensor.matmul(
                        po[:, cs], lhsT=w2t[:, b * E:b * E + 128],
                        rhs=h0[:, cs], start=True, stop=False)
                    nc.tensor.matmul(
                        po[:, cs], lhsT=w2t[:, b * E + 128:b * E + 256],
                        rhs=h1[:, cs], start=False, stop=False)
                    nc.tensor.matmul(
                        po[:, cs], lhsT=bt[0:1, b * (E + D) + 256:(b + 1) * (E + D)],
                        rhs=ones[0:1, :], start=False, stop=True)

                osb = opool.tile([128, B * Cr], f32, tag="osb")
                nc.vector.tensor_copy(osb, po)
                nc.sync.dma_start(
                    out=outT[:, j * B * Cr:(j + 1) * B * Cr], in_=osb)

    nc.compile()
    return nc


def _plan(rules: np.ndarray, R: int):
    """Sort tokens by rule, build fixed-capacity slots, deal to cores."""
    order = np.argsort(rules, kind="stable")
    counts = np.bincount(rules, minlength=R)
    starts = np.concatenate([[0], np.cumsum(counts)])

    Cr = int(max(8, counts.max()))
    Cr = (Cr + 3) // 4 * 4
    Cr = min(Cr, 512)
    for Bc in (16, 8, 4, 2, 1):
        if Bc * Cr <= 512:
            B = Bc
            break

    slots = []  # (sorted_start, length)
    for r in range(R):
        c = int(counts[r])
        s = int(starts[r])
        if c == 0:
            slots.append((s, 0))
        else:
            off = 0
            while off < c:
                ln = min(Cr, c - off)
                slots.append((s + off, ln))
                off += ln
    # rule id per slot for the weight gather
    slot_rules = []
    for r in range(R):
        c = int(counts[r])
        n = max(1, -(-c // Cr))
        slot_rules.extend([r] * n)

    S = len(slots)
    K_c = -(-S // (N_CORES * B)) * B  # slots per core, multiple of B
    total = K_c * N_CORES
    slots += [(0, 0)] * (total - S)
    slot_rules += [0] * (total - S)
    return order, np.array(slot_rules), slots, K_c, Cr, B


def _prepare(x, rules, w1, b1, w2, b2):
    x = np.ascontiguousarray(np.asarray(x), dtype=np.float32)
    rules = np.asarray(rules).astype(np.int64)
    w1 = np.ascontiguousarray(np.asarray(w1), dtype=np.float32)
    b1 = np.ascontiguousarray(np.asarray(b1), dtype=np.float32)
    w2 = np.ascontiguousarray(np.asarray(w2), dtype=np.float32)
    b2 = np.ascontiguousarray(np.asarray(b2), dtype=np.float32)

    R = w1.shape[0]
    order, slot_rules, slots, K_c, Cr, B = _plan(rules, R)

    bcat = np.concatenate([b1, b2], axis=1)  # [R, E+D]

    in_maps = []
    for c in range(N_CORES):
        sl = slice(c * K_c, (c + 1) * K_c)
        sr = slot_rules[sl]
        xT = np.zeros((D, K_c * Cr), dtype=np.float32)
        for k, (s, ln) in enumerate(slots[sl.start:sl.stop]):
            if ln:
                xT[:, k * Cr:k * Cr + ln] = x[order[s:s + ln]].T
        in_maps.append({
            "xT": xT,
            "w1": np.ascontiguousarray(w1[sr]),
            "w2": np.ascontiguousarray(w2[sr]),
            "bias": np.ascontiguousarray(bcat[sr]),
        })
    return in_maps, order, slots, K_c, Cr, B


def _unpack(res, order, slots, K_c, Cr, N):
    out = np.empty((N, D), dtype=np.float32)
    for c in range(N_CORES):
        outT = res.results[c]["outT"]
        for k, (s, ln) in enumerate(slots[c * K_c:(c + 1) * K_c]):
            if ln:
                out[order[s:s + ln]] = outT[:, k * Cr:k * Cr + ln].T
    return out


def kernel(x, rules, w1, b1, w2, b2):
    N = np.asarray(x).shape[0]
    in_maps, order, slots, K_c, Cr, B = _prepare(x, rules, w1, b1, w2, b2)
    nc = _build_nc(K_c, Cr, B)
    res = run_bass_kernel_spmd(nc, in_maps, list(range(N_CORES)))
    return _unpack(res, order, slots, K_c, Cr, N)


# revision 8
# speedup vs baseline: 1.6933x; 1.6933x over previous
"""Batched rule-expert FFN (MoE routing) on 8 Trainium2 NeuronCores.

Strategy (expert/slot parallel with host-side dispatch):
  - Sort tokens by rule id on the host; each rule's tokens form one "slot"
    (rules with more than `Cr` tokens get several slots, zero-hit rules get
    an empty slot so the device schedule stays fully static).
  - Slots are dealt contiguously to the 8 cores (128 slots/core for 1024
    rules).  For each core the host gathers that core's slot weights
    w1[rule], w2[rule], biases, plus an x^T buffer [128, K_c*Cr] whose
    column block k*Cr:(k+1)*Cr holds the (transposed, zero-padded) tokens
    of slot k.
  - The device kernel is a static loop over slot-blocks: load w1/w2/bias
    blocks, per slot run  H^T = gelu(W1^T X^T + b1),  Out^T = W2^T H^T + b2
    with tokens in the free (moving) dimension, biases folded into the
    PSUM accumulation as K=1 matmuls against a ones row.
  - Host scatters Out^T columns back to token order.

Traffic per core ~= 32 MiB of expert tables (+ ~4 MiB padded x/out), which
is the memory roofline for this problem: every rule is hit with very high
probability, so the whole [R,D,E]+[R,E,D] table must be read exactly once.
"""

import numpy as np

import concourse.bass as bass
import concourse.bacc as bacc
import concourse.mybir as mybir
from concourse.tile import TileContext
from concourse.bass_utils import run_bass_kernel_spmd

N_CORES = 8
D = 128   # d_model
E = 256   # expert dim
EC = E // 128  # e-chunks of 128 partitions


def _build_nc(K_c: int, Cr: int, B: int):
    """Bass program for one core: K_c slots of capacity Cr, B slots/block."""
    f32 = mybir.dt.float32
    nc = bacc.Bacc("TRN2", target_bir_lowering=False)

    xT = nc.declare_dram_parameter("xT", [D, K_c * Cr], f32, isOutput=False)
    w1 = nc.declare_dram_parameter("w1", [K_c, D, E], f32, isOutput=False)
    w2 = nc.declare_dram_parameter("w2", [K_c, E, D], f32, isOutput=False)
    bb = nc.declare_dram_parameter("bias", [K_c, E + D], f32, isOutput=False)
    outT = nc.declare_dram_parameter("outT", [D, K_c * Cr], f32, isOutput=True)

    nblk = K_c // B
    gelu = mybir.ActivationFunctionType.Gelu
    NB = (E + D) // 128  # bias chunks per rule (b1c0, b1c1, b2)

    with TileContext(nc) as tc:
        with (
            tc.tile_pool(name="wpool", bufs=3) as wpool,
            tc.tile_pool(name="xpool", bufs=3) as xpool,
            tc.tile_pool(name="bpool", bufs=3) as bpool,
            tc.tile_pool(name="hpool", bufs=2) as hpool,
            tc.tile_pool(name="opool", bufs=3) as opool,
            tc.tile_pool(name="ppool", bufs=2, space="PSUM") as ppool,
        ):
            for j in range(nblk):
                w1t = wpool.tile([128, B * E], f32, tag="w1t")
                nc.sync.dma_start(
                    out=w1t.rearrange("p (r e) -> p r e", e=E),
                    in_=w1[j * B:(j + 1) * B].rearrange("r d e -> d r e"))
                w2t = wpool.tile([128, B * E], f32, tag="w2t")
                nc.sync.dma_start(
                    out=w2t.rearrange("p (r c d) -> p r c d", c=EC, d=128),
                    in_=w2[j * B:(j + 1) * B].rearrange(
                        "r (c p) d -> p r c d", p=128))
                # bias tile [128, B*3]: col b*3+c = chunk c of rule b
                # (c=0,1 -> b1 halves; c=2 -> b2), partition = feature
                bt = bpool.tile([128, B * NB], f32, tag="bt")
                nc.sync.dma_start(
                    out=bt.rearrange("p (r c) -> p r c", c=NB),
                    in_=bb[j * B:(j + 1) * B].rearrange(
                        "r (c p) -> p r c", p=128))
                xt = xpool.tile([128, B * Cr], f32, tag="xt")
                nc.sync.dma_start(
                    out=xt, in_=xT[:, j * B * Cr:(j + 1) * B * Cr])

                ph0 = ppool.tile([128, B * Cr], f32, tag="ph0")
                ph1 = ppool.tile([128, B * Cr], f32, tag="ph1")
                po = ppool.tile([128, B * Cr], f32, tag="po")
                h0 = hpool.tile([128, B * Cr], f32, tag="h0")
                h1 = hpool.tile([128, B * Cr], f32, tag="h1")
                osb = opool.tile([128, B * Cr], f32, tag="osb")

                # ---- layer 1: H^T[e, tok] = gelu(W1^T X^T + b1) ---------
                for b in range(B):
                    cs = slice(b * Cr, (b + 1) * Cr)
                    nc.tensor.matmul(
                        ph0[:, cs], lhsT=w1t[:, b * E:b * E + 128],
                        rhs=xt[:, cs], start=True, stop=True)
                    nc.tensor.matmul(
                        ph1[:, cs], lhsT=w1t[:, b * E + 128:b * E + 256],
                        rhs=xt[:, cs], start=True, stop=True)
                    nc.scalar.activation(
                        h0[:, cs], ph0[:, cs], gelu,
                        bias=bt[:, b * NB:b * NB + 1])
                    nc.scalar.activation(
                        h1[:, cs], ph1[:, cs], gelu,
                        bias=bt[:, b * NB + 1:b * NB + 2])

                # ---- layer 2: Out^T[d, tok] = W2^T H^T + b2 -------------
                for b in range(B):
                    cs = slice(b * Cr, (b + 1) * Cr)
                    nc.tensor.matmul(
                        po[:, cs], lhsT=w2t[:, b * E:b * E + 128],
                        rhs=h0[:, cs], start=True, stop=False)
                    nc.tensor.matmul(
                        po[:, cs], lhsT=w2t[:, b * E + 128:b * E + 256],
                        rhs=h1[:, cs], start=False, stop=True)
                    nc.vector.tensor_scalar_add(
                        osb[:, cs], po[:, cs],
                        bt[:, b * NB + 2:b * NB + 3])

                nc.sync.dma_start(
                    out=outT[:, j * B * Cr:(j + 1) * B * Cr], in_=osb)

    nc.compile()
    return nc


def _plan(rules: np.ndarray, R: int):
    """Sort tokens by rule, build fixed-capacity slots, deal to cores."""
    order = np.argsort(rules, kind="stable")
    counts = np.bincount(rules, minlength=R)
    starts = np.concatenate([[0], np.cumsum(counts)])

    Cr = int(max(8, counts.max()))
    Cr = (Cr + 3) // 4 * 4
    Cr = min(Cr, 512)
    for Bc in (16, 8, 4, 2, 1):
        if Bc * Cr <= 512:
            B = Bc
            break

    slots = []  # (sorted_start, length)
    for r in range(R):
        c = int(counts[r])
        s = int(starts[r])
        if c == 0:
            slots.append((s, 0))
        else:
            off = 0
            while off < c:
                ln = min(Cr, c - off)
                slots.append((s + off, ln))
                off += ln
    # rule id per slot for the weight gather
    slot_rules = []
    for r in range(R):
        c = int(counts[r])
        n = max(1, -(-c // Cr))
        slot_rules.extend([r] * n)

    S = len(slots)
    K_c = -(-S // (N_CORES * B)) * B  # slots per core, multiple of B
    total = K_c * N_CORES
    slots += [(0, 0)] * (total - S)
    slot_rules += [0] * (total - S)
    return order, np.array(slot_rules), slots, K_c, Cr, B


def _prepare(x, rules, w1, b1, w2, b2):
    x = np.ascontiguousarray(np.asarray(x), dtype=np.float32)
    rules = np.asarray(rules).astype(np.int64)
    w1 = np.ascontiguousarray(np.asarray(w1), dtype=np.float32)
    b1 = np.ascontiguousarray(np.asarray(b1), dtype=np.float32)
    w2 = np.ascontiguousarray(np.asarray(w2), dtype=np.float32)
    b2 = np.ascontiguousarray(np.asarray(b2), dtype=np.float32)

    R = w1.shape[0]
    order, slot_rules, slots, K_c, Cr, B = _plan(rules, R)

    bcat = np.concatenate([b1, b2], axis=1)  # [R, E+D]

    in_maps = []
    for c in range(N_CORES):
        sl = slice(c * K_c, (c + 1) * K_c)
        sr = slot_rules[sl]
        xT = np.zeros((D, K_c * Cr), dtype=np.float32)
        for k, (s, ln) in enumerate(slots[sl.start:sl.stop]):
            if ln:
                xT[:, k * Cr:k * Cr + ln] = x[order[s:s + ln]].T
        in_maps.append({
            "xT": xT,
            "w1": np.ascontiguousarray(w1[sr]),
            "w2": np.ascontiguousarray(w2[sr]),
            "bias": np.ascontiguousarray(bcat[sr]),
        })
    return in_maps, order, slots, K_c, Cr, B


def _unpack(res, order, slots, K_c, Cr, N):
    out = np.empty((N, D), dtype=np.float32)
    for c in range(N_CORES):
        outT = res.results[c]["outT"]
        for k, (s, ln) in enumerate(slots[c * K_c:(c + 1) * K_c]):
            if ln:
                out[order[s:s + ln]] = outT[:, k * Cr:k * Cr + ln].T
    return out


def kernel(x, rules, w1, b1, w2, b2):
    N = np.asarray(x).shape[0]
    in_maps, order, slots, K_c, Cr, B = _prepare(x, rules, w1, b1, w2, b2)
    nc = _build_nc(K_c, Cr, B)
    res = run_bass_kernel_spmd(nc, in_maps, list(range(N_CORES)))
    return _unpack(res, order, slots, K_c, Cr, N)


# revision 15
# speedup vs baseline: 3.4893x; 2.0607x over previous
"""Batched rule-expert FFN (MoE routing) on 8 Trainium2 NeuronCores.

Strategy (expert/slot parallel with host-side dispatch):
  - Sort tokens by rule id on the host; each rule's tokens form one "slot"
    (rules with more than `Cr` tokens get several slots, zero-hit rules get
    an empty slot so the device schedule stays fully static).
  - Slots are dealt contiguously to the 8 cores (128 slots/core for 1024
    rules).  For each core the host gathers that core's slot weights
    w1[rule], w2[rule], biases, plus an x^T buffer [128, K_c*Cr] whose
    column block k*Cr:(k+1)*Cr holds the (transposed, zero-padded) tokens
    of slot k.
  - The device kernel is a static loop over slot-blocks: load w1/w2/bias
    blocks, per slot run  H^T = gelu(W1^T X^T + b1),  Out^T = W2^T H^T + b2
    with tokens in the free (moving) dimension, biases folded into the
    PSUM accumulation as K=1 matmuls against a ones row.
  - Host scatters Out^T columns back to token order.

Traffic per core ~= 32 MiB of expert tables (+ ~4 MiB padded x/out), which
is the memory roofline for this problem: every rule is hit with very high
probability, so the whole [R,D,E]+[R,E,D] table must be read exactly once.
"""

import numpy as np

import concourse.bass as bass
import concourse.bacc as bacc
import concourse.mybir as mybir
from concourse.tile import TileContext
from concourse.bass_utils import run_bass_kernel_spmd

N_CORES = 8
D = 128   # d_model
E = 256   # expert dim
EC = E // 128  # e-chunks of 128 partitions


def _build_nc(K_c: int, Cr: int, B: int, mm_dt: str = "float32"):
    """Bass program for one core: K_c slots of capacity Cr, B slots/block."""
    f32 = mybir.dt.float32
    nc = bacc.Bacc("TRN2", target_bir_lowering=False)

    if mm_dt == "float32r":
        mdt = f32
        cast = lambda ap: ap.bitcast(mybir.dt.float32r)  # noqa: E731
    elif mm_dt == "bfloat16":
        mdt = mybir.dt.bfloat16
        cast = lambda ap: ap  # noqa: E731
    else:
        mdt = f32
        cast = lambda ap: ap  # noqa: E731

    xT = nc.declare_dram_parameter("xT", [D, K_c * Cr], mdt, isOutput=False)
    w1 = nc.declare_dram_parameter("w1", [K_c, D, E], mdt, isOutput=False)
    w2 = nc.declare_dram_parameter("w2", [K_c, E, D], mdt, isOutput=False)
    bb = nc.declare_dram_parameter("bias", [K_c, E + D], f32, isOutput=False)
    outT = nc.declare_dram_parameter("outT", [D, K_c * Cr], f32, isOutput=True)

    nblk = K_c // B
    gelu = mybir.ActivationFunctionType.Gelu
    NB = (E + D) // 128  # bias chunks per rule (b1c0, b1c1, b2)

    with TileContext(nc) as tc:
        with (
            tc.tile_pool(name="wpool", bufs=3) as wpool,
            tc.tile_pool(name="xpool", bufs=3) as xpool,
            tc.tile_pool(name="bpool", bufs=3) as bpool,
            tc.tile_pool(name="hpool", bufs=2) as hpool,
            tc.tile_pool(name="opool", bufs=3) as opool,
            tc.tile_pool(name="ppool", bufs=2, space="PSUM") as ppool,
        ):
            for j in range(nblk):
                w1t = wpool.tile([128, B * E], mdt, tag="w1t")
                nc.sync.dma_start(
                    out=w1t.rearrange("p (r e) -> p r e", e=E),
                    in_=w1[j * B:(j + 1) * B].rearrange("r d e -> d r e"))
                w2t = wpool.tile([128, B * E], mdt, tag="w2t")
                nc.sync.dma_start(
                    out=w2t.rearrange("p (r c d) -> p r c d", c=EC, d=128),
                    in_=w2[j * B:(j + 1) * B].rearrange(
                        "r (c p) d -> p r c d", p=128))
                # bias tile [128, B*3]: col b*3+c = chunk c of rule b
                # (c=0,1 -> b1 halves; c=2 -> b2), partition = feature
                bt = bpool.tile([128, B * NB], f32, tag="bt")
                nc.sync.dma_start(
                    out=bt.rearrange("p (r c) -> p r c", c=NB),
                    in_=bb[j * B:(j + 1) * B].rearrange(
                        "r (c p) -> p r c", p=128))
                xt = xpool.tile([128, B * Cr], mdt, tag="xt")
                nc.sync.dma_start(
                    out=xt, in_=xT[:, j * B * Cr:(j + 1) * B * Cr])

                ph0 = ppool.tile([128, B * Cr], f32, tag="ph0")
                ph1 = ppool.tile([128, B * Cr], f32, tag="ph1")
                po = ppool.tile([128, B * Cr], f32, tag="po")
                h0 = hpool.tile([128, B * Cr], mdt, tag="h0")
                h1 = hpool.tile([128, B * Cr], mdt, tag="h1")
                osb = opool.tile([128, B * Cr], f32, tag="osb")

                # ---- layer 1: H^T[e, tok] = gelu(W1^T X^T + b1) ---------
                for b in range(B):
                    cs = slice(b * Cr, (b + 1) * Cr)
                    nc.tensor.matmul(
                        ph0[:, cs], lhsT=cast(w1t[:, b * E:b * E + 128]),
                        rhs=cast(xt[:, cs]), start=True, stop=True)
                    nc.tensor.matmul(
                        ph1[:, cs], lhsT=cast(w1t[:, b * E + 128:b * E + 256]),
                        rhs=cast(xt[:, cs]), start=True, stop=True)
                    nc.scalar.activation(
                        h0[:, cs], ph0[:, cs], gelu,
                        bias=bt[:, b * NB:b * NB + 1])
                    nc.scalar.activation(
                        h1[:, cs], ph1[:, cs], gelu,
                        bias=bt[:, b * NB + 1:b * NB + 2])

                # ---- layer 2: Out^T[d, tok] = W2^T H^T + b2 -------------
                for b in range(B):
                    cs = slice(b * Cr, (b + 1) * Cr)
                    nc.tensor.matmul(
                        po[:, cs], lhsT=cast(w2t[:, b * E:b * E + 128]),
                        rhs=cast(h0[:, cs]), start=True, stop=False)
                    nc.tensor.matmul(
                        po[:, cs], lhsT=cast(w2t[:, b * E + 128:b * E + 256]),
                        rhs=cast(h1[:, cs]), start=False, stop=True)
                    nc.vector.tensor_scalar_add(
                        osb[:, cs], po[:, cs],
                        bt[:, b * NB + 2:b * NB + 3])

                nc.sync.dma_start(
                    out=outT[:, j * B * Cr:(j + 1) * B * Cr], in_=osb)

    nc.compile()
    return nc


def _plan(rules: np.ndarray, R: int):
    """Sort tokens by rule, build fixed-capacity slots, deal to cores."""
    order = np.argsort(rules, kind="stable")
    counts = np.bincount(rules, minlength=R)
    starts = np.concatenate([[0], np.cumsum(counts)])

    Cr = int(max(8, counts.max()))
    Cr = (Cr + 3) // 4 * 4
    Cr = min(Cr, 512)
    for Bc in (16, 8, 4, 2, 1):
        if Bc * Cr <= 512:
            B = Bc
            break

    slots = []  # (sorted_start, length)
    for r in range(R):
        c = int(counts[r])
        s = int(starts[r])
        if c == 0:
            slots.append((s, 0))
        else:
            off = 0
            while off < c:
                ln = min(Cr, c - off)
                slots.append((s + off, ln))
                off += ln
    # rule id per slot for the weight gather
    slot_rules = []
    for r in range(R):
        c = int(counts[r])
        n = max(1, -(-c // Cr))
        slot_rules.extend([r] * n)

    S = len(slots)
    K_c = -(-S // (N_CORES * B)) * B  # slots per core, multiple of B
    total = K_c * N_CORES
    slots += [(0, 0)] * (total - S)
    slot_rules += [0] * (total - S)
    return order, np.array(slot_rules), slots, K_c, Cr, B


MM_DT = "float32"  # matmul-operand dtype: "float32" or "bfloat16"


def _prepare(x, rules, w1, b1, w2, b2, mm_dt=MM_DT):
    x = np.ascontiguousarray(np.asarray(x), dtype=np.float32)
    rules = np.asarray(rules).astype(np.int64)
    w1 = np.ascontiguousarray(np.asarray(w1), dtype=np.float32)
    b1 = np.ascontiguousarray(np.asarray(b1), dtype=np.float32)
    w2 = np.ascontiguousarray(np.asarray(w2), dtype=np.float32)
    b2 = np.ascontiguousarray(np.asarray(b2), dtype=np.float32)

    R = w1.shape[0]
    order, slot_rules, slots, K_c, Cr, B = _plan(rules, R)

    bcat = np.concatenate([b1, b2], axis=1)  # [R, E+D]

    if mm_dt == "bfloat16":
        import ml_dtypes
        mnp = ml_dtypes.bfloat16
    else:
        mnp = np.float32

    in_maps = []
    for c in range(N_CORES):
        sl = slice(c * K_c, (c + 1) * K_c)
        sr = slot_rules[sl]
        xT = np.zeros((D, K_c * Cr), dtype=mnp)
        for k, (s, ln) in enumerate(slots[sl.start:sl.stop]):
            if ln:
                xT[:, k * Cr:k * Cr + ln] = x[order[s:s + ln]].T.astype(mnp)
        in_maps.append({
            "xT": xT,
            "w1": np.ascontiguousarray(w1[sr].astype(mnp)),
            "w2": np.ascontiguousarray(w2[sr].astype(mnp)),
            "bias": np.ascontiguousarray(bcat[sr]),
        })
    return in_maps, order, slots, K_c, Cr, B


def _unpack(res, order, slots, K_c, Cr, N):
    out = np.empty((N, D), dtype=np.float32)
    for c in range(N_CORES):
        outT = res.results[c]["outT"]
        for k, (s, ln) in enumerate(slots[c * K_c:(c + 1) * K_c]):
            if ln:
                out[order[s:s + ln]] = outT[:, k * Cr:k * Cr + ln].T
    return out


def kernel(x, rules, w1, b1, w2, b2):
    N = np.asarray(x).shape[0]
    in_maps, order, slots, K_c, Cr, B = _prepare(
        x, rules, w1, b1, w2, b2, mm_dt=MM_DT)
    nc = _build_nc(K_c, Cr, B, mm_dt=MM_DT)
    res = run_bass_kernel_spmd(nc, in_maps, list(range(N_CORES)))
    return _unpack(res, order, slots, K_c, Cr, N)
